# revision 1
# baseline (speedup 1.0000x reference)
"""Trainium2 Bass kernel for segmented ("sparse") attention.

Reference math (per batch of 16, S=1024, H=441):
  q = hs @ Wq + bq ; k = hs @ Wk + bk ; v = hs @ Wv + bv
  for each of 9 column segments [s,e): softmax(q_i k_i^T / sqrt(d_i)) @ v_i
  out = concat(ctx_i) @ Wo + bo

Sharding: pure data parallel over batch, 2 batches per NeuronCore x 8 cores.

Device-side strategy (per batch):
  - hs pre-transposed on host: hsT [441, 1024] (h on partitions)
  - qT,kT [441, 1024] projections stored in a packed layout of 5 tiles where
    each segment's rows sit at a 32-aligned partition base
  - scoresT[t, s] = kT_seg[:, tcol].T @ qT_seg -> PSUM [128t, 512s]
  - E = exp(scoresT / sqrt(d)) on the scalar engine, PSUM -> SBUF
  - v in natural layout [s, 441] with a ones column appended per segment
    (aug layout): one accumulated matmul over the 8 t-chunks yields both
    ctx_unnorm^T and the softmax denominator
  - normalize: denom row DMA-gathered, reciprocal on DVE, broadcast across
    partitions with a K=1 ones matmul, multiplied into packed ctxT
  - outT[ho, s] = Wo_packed.T @ ctxT (+bo), accumulated per base-partition
    group (fp32r groups must not mix lhsT base partitions) and summed on DVE
  - host transposes outT back to [S, 441]

All matmuls are float32r (TF32-like). fp32r oddities honored: moving/dst
free sizes even, out base partition 0, uniform base per accumulation group.
"""

import os
import math
import numpy as np
from contextlib import ExitStack

import concourse.bacc as bacc
import concourse.tile as tile
import concourse.mybir as mybir
from concourse.bass_utils import run_bass_kernel_spmd

F32 = mybir.dt.float32
F32R = mybir.dt.float32r
BF16 = mybir.dt.bfloat16
AF = mybir.ActivationFunctionType

HID = 441
HID2 = HID + 1  # even-padded weight width
S = 1024
B = 16
N_CORES = 8
BPC = B // N_CORES
BOUNDS = [0, 7, 21, 49, 105, 161, 217, 273, 357, 441]
NSEG = 9
DSEG = [BOUNDS[i + 1] - BOUNDS[i] for i in range(NSEG)]
NHC = 4
HCH = [(i * 128, min(128, HID - i * 128)) for i in range(NHC)]
NTC = 8
SH = 512
NPT = 5

# Packed row layout: (seg, off_within_seg, length, pack_tile, pack_base)
# One piece per segment; bases 32-aligned; base 96 passed as explicit
# tile_position (auto-derive only allows 0/32/64).
PIECES = [
    (0, 0, 7, 4, 0),
    (1, 0, 14, 1, 96),
    (2, 0, 28, 0, 96),
    (3, 0, 56, 2, 0),
    (4, 0, 56, 3, 0),
    (5, 0, 56, 2, 64),
    (6, 0, 56, 3, 64),
    (7, 0, 84, 0, 0),
    (8, 0, 84, 1, 0),
]
SEG_PIECE = {p[0]: p for p in PIECES}

# output projection: one PSUM accumulation group per lhsT base partition
WO_GROUPS = [[p for p in PIECES if p[4] == b] for b in (0, 64, 96)]

# v augmented layout: per segment [v columns (d), ones column]
AUG_OFF = [BOUNDS[i] + i for i in range(NSEG)]
AUG_W = HID + NSEG  # 450


def _repack_jobs():
    jobs = []
    for seg, off, length, pt, pb in PIECES:
        g0 = BOUNDS[seg] + off
        done = 0
        while done < length:
            g = g0 + done
            ac = g // 128
            take = min(length - done, (ac + 1) * 128 - g)
            jobs.append((ac, g - ac * 128, pt, pb + done, take))
            done += take
    return jobs


REPACK = _repack_jobs()

_CACHE = {}


def _build():
    nc = bacc.Bacc("TRN2", target_bir_lowering=False, debug=False)

    hsT = nc.dram_tensor("hsT", [BPC, HID, S], BF16, kind="ExternalInput").ap()
    Wq_d = nc.dram_tensor("Wq", [HID, HID2], BF16, kind="ExternalInput").ap()
    Wk_d = nc.dram_tensor("Wk", [HID, HID2], BF16, kind="ExternalInput").ap()
    Wv_d = nc.dram_tensor("Wv", [HID, HID2], BF16, kind="ExternalInput").ap()
    Wo_d = nc.dram_tensor("Wop", [NPT, 128, HID2], BF16, kind="ExternalInput").ap()
    bq_d = nc.dram_tensor("bq", [HID, 1], F32, kind="ExternalInput").ap()
    bk_d = nc.dram_tensor("bk", [HID, 1], F32, kind="ExternalInput").ap()
    bo_d = nc.dram_tensor("bo", [HID, 1], F32, kind="ExternalInput").ap()
    bvb_d = nc.dram_tensor("bvb", [128, HID], F32, kind="ExternalInput").ap()
    ind9_d = nc.dram_tensor("ind9", [NSEG, NSEG * 128], F32R, kind="ExternalInput").ap()
    outT = nc.dram_tensor("outT", [BPC, HID, S], F32, kind="ExternalOutput").ap()

    with tile.TileContext(nc) as tc, ExitStack() as ctx, nc.allow_low_precision(
        reason="float32r rounding for matmul inputs"
    ):
        cpool = ctx.enter_context(tc.tile_pool(name="c", bufs=1))
        hpool = ctx.enter_context(tc.tile_pool(name="h", bufs=1))
        apool = ctx.enter_context(tc.tile_pool(name="a", bufs=1))
        ppool = ctx.enter_context(tc.tile_pool(name="p", bufs=1))
        epool = ctx.enter_context(tc.tile_pool(name="e", bufs=2))
        vpool = ctx.enter_context(tc.tile_pool(name="v", bufs=18))
        spool = ctx.enter_context(tc.tile_pool(name="s", bufs=2))
        ps = ctx.enter_context(tc.tile_pool(name="ps", bufs=1, space="PSUM"))

        # ---- constants ----
        Wq_sb, Wk_sb, Wv_sb = [], [], []
        for hc, (h0, hw) in enumerate(HCH):
            for lst, src, nm in ((Wq_sb, Wq_d, "wq"), (Wk_sb, Wk_d, "wk"), (Wv_sb, Wv_d, "wv")):
                t = cpool.tile([hw, HID2], BF16, name=f"{nm}{hc}", tag=f"{nm}{hc}")
                nc.sync.dma_start(out=t, in_=src[h0 : h0 + hw, :])
                lst.append(t)
        Wo_sb = []
        for i in range(NPT):
            t = cpool.tile([128, HID2], BF16, name=f"wo{i}", tag=f"wo{i}")
            nc.sync.dma_start(out=t, in_=Wo_d[i])
            Wo_sb.append(t)
        bq_sb, bk_sb, bo_sb = [], [], []
        for hc, (h0, hw) in enumerate(HCH):
            for lst, src, nm in ((bq_sb, bq_d, "bq"), (bk_sb, bk_d, "bk"), (bo_sb, bo_d, "bo")):
                t = cpool.tile([hw, 1], F32, name=f"{nm}{hc}", tag=f"{nm}{hc}")
                nc.sync.dma_start(out=t, in_=src[h0 : h0 + hw, :])
                lst.append(t)
        bvb = cpool.tile([128, HID], F32, name="bvb", tag="bvb")
        nc.sync.dma_start(out=bvb, in_=bvb_d)
        ind9 = cpool.tile([NSEG, NSEG * 128], F32R, name="ind9", tag="ind9")
        nc.sync.dma_start(out=ind9, in_=ind9_d)

        for b in range(BPC):
            # ---- load hsT ----
            hs = []
            for hc, (h0, hw) in enumerate(HCH):
                t = hpool.tile([hw, S], BF16, name=f"hs{hc}", tag=f"hs{hc}", bufs=2)
                nc.sync.dma_start(out=t, in_=hsT[b, h0 : h0 + hw, :])
                hs.append(t)

            # ---- q/k projections into aligned chunks, DMA-repacked ----
            packs = {}
            for nm, W_sb, b_sb in (("q", Wq_sb, bq_sb), ("k", Wk_sb, bk_sb)):
                al = []
                for mc, (m0, mw) in enumerate(HCH):
                    qa = apool.tile([mw, S], BF16, name=f"al{nm}{mc}", tag=f"al{mc}", bufs=2)
                    for half in range(2):
                        pa = ps.tile([128, SH], F32, name=f"pp{nm}{mc}{half}", tag="x", bufs=2)
                        for hc, (h0, hw) in enumerate(HCH):
                            nc.tensor.matmul(
                                pa[0:mw, :],
                                W_sb[hc][:, m0 : m0 + mw],
                                hs[hc][:, half * SH : (half + 1) * SH],
                                start=(hc == 0),
                                stop=(hc == NHC - 1),
                            )
                        nc.vector.tensor_scalar_add(
                            qa[:, half * SH : (half + 1) * SH], pa[0:mw, :], b_sb[mc][:]
                        )
                    al.append(qa)
                pk = []
                for i in range(NPT):
                    t = ppool.tile([128, S], BF16, name=f"{nm}T{i}", tag=f"{nm}T{i}", bufs=2)
                    pk.append(t)
                for ac, r0, pt, pb, ln in REPACK:
                    nc.gpsimd.dma_start(out=pk[pt][pb : pb + ln, :], in_=al[ac][r0 : r0 + ln, :])
                packs[nm] = pk
            qT, kT = packs["q"], packs["k"]

            # ---- v projection (natural layout) + aug ones columns ----
            vaug = []
            for sc in range(NTC):
                pv = ps.tile([128, HID2], F32, name=f"pv{sc}", tag="x", bufs=2)
                for hc, (h0, hw) in enumerate(HCH):
                    nc.tensor.matmul(
                        pv[:],
                        hs[hc][:, sc * 128 : (sc + 1) * 128],
                        Wv_sb[hc][:],
                        start=(hc == 0),
                        stop=(hc == NHC - 1),
                    )
                vt = spool.tile([128, HID], BF16, name=f"vt{sc}", tag="vt")
                nc.vector.tensor_add(vt[:], pv[:, 0:HID], bvb[:])
                va = vpool.tile([128, AUG_W], BF16, name=f"va{sc}", tag="va")
                for sg in range(NSEG):
                    s0, s1 = BOUNDS[sg], BOUNDS[sg + 1]
                    a0 = AUG_OFF[sg]
                    nc.sync.dma_start(out=va[:, a0 : a0 + (s1 - s0)], in_=vt[:, s0:s1])
                    nc.vector.memset(va[:, a0 + (s1 - s0) : a0 + (s1 - s0) + 1], 1.0)
                vaug.append(va)

            # ---- attention ----
            cxT = [ppool.tile([128, S], BF16, name=f"cxT{i}", tag=f"cxT{i}") for i in range(NPT)]
            for half in range(2):
                hsl = slice(half * SH, (half + 1) * SH)
                us = [
                    spool.tile([96, SH], BF16, name=f"u{sg}", tag=f"u{sg}", bufs=1)
                    for sg in range(NSEG)
                ]
                den9 = spool.tile([NSEG, SH], F32, name="den9", tag="den9", bufs=1)
                for sg in range(NSEG):
                    d = DSEG[sg]
                    scale = 1.0 / math.sqrt(d)
                    _, off, ln, pt, pb = SEG_PIECE[sg]
                    tp = (96, 0) if pb == 96 else None
                    E = epool.tile([128, NTC * SH], BF16, name=f"E{sg}", tag="E")
                    for t2 in range(NTC // 2):
                        pm = ps.tile([128, 2 * SH], F32, name=f"pm{sg}{t2}", tag="m1", bufs=2)
                        for k2 in range(2):
                            t = 2 * t2 + k2
                            nc.tensor.matmul(
                                pm[:, k2 * SH : (k2 + 1) * SH],
                                kT[pt][pb : pb + ln, t * 128 : (t + 1) * 128],
                                qT[pt][pb : pb + ln, hsl],
                                start=True,
                                stop=True,
                                tile_position=tp,
                            )
                        nc.scalar.activation(
                            E[:, t2 * 2 * SH : (t2 + 1) * 2 * SH], pm[:], AF.Exp, scale=scale
                        )
                    # ctx_unnorm^T and denominator via aug ones column
                    pu = ps.tile([128, SH], F32, name=f"pu{sg}", tag="m2", bufs=2)
                    a0 = AUG_OFF[sg]
                    for t in range(NTC):
                        nc.tensor.matmul(
                            pu[0 : d + 1, :],
                            vaug[t][:, a0 : a0 + d + 1],
                            E[:, t * SH : (t + 1) * SH],
                            start=(t == 0),
                            stop=(t == NTC - 1),
                        )
                    u = us[sg]
                    nc.vector.tensor_copy(u[0 : d + 1, :], pu[0 : d + 1, :])
                    nc.gpsimd.dma_start(out=den9[sg : sg + 1, :], in_=u[d : d + 1, :])
                # batched reciprocal of all 9 denominators, then per-seg
                # partition-broadcast via indicator matmul
                rec9 = spool.tile([NSEG, SH], F32, name="rec9", tag="rec9", bufs=1)
                scr9 = spool.tile([NSEG, SH], F32, name="scr9", tag="scr9", bufs=1)
                nc.vector.reciprocal_approx_accurate(rec9[:], den9[:], scratch=scr9[:])
                rec9r = spool.tile([NSEG, SH], F32R, name="rec9r", tag="rec9r", bufs=1)
                nc.vector.tensor_copy(rec9r[:], rec9[:])
                for sg in range(NSEG):
                    d = DSEG[sg]
                    _, off, ln, pt, pb = SEG_PIECE[sg]
                    pb_ps = ps.tile([128, SH], F32, name=f"pb{sg}", tag="x", bufs=2)
                    nc.tensor.matmul(
                        pb_ps[0:d, :], ind9[:, sg * 128 : sg * 128 + d], rec9r[:],
                        start=True, stop=True,
                    )
                    bc = spool.tile([96, SH], BF16, name=f"bc{sg}", tag="bc")
                    nc.vector.tensor_copy(bc[0:d, :], pb_ps[0:d, :])
                    nc.vector.tensor_mul(
                        cxT[pt][pb : pb + d, hsl], us[sg][0:d, :], bc[0:d, :]
                    )

                # ---- output projection for this half ----
                for hc, (h0, hw) in enumerate(HCH):
                    pos = []
                    for g, plist in enumerate(WO_GROUPS):
                        po = ps.tile([128, SH], F32, name=f"po{hc}{g}", tag="m2", bufs=2)
                        for j, (_, off, ln, pt, pb) in enumerate(plist):
                            nc.tensor.matmul(
                                po[0:hw, :],
                                Wo_sb[pt][pb : pb + ln, h0 : h0 + hw],
                                cxT[pt][pb : pb + ln, hsl],
                                start=(j == 0),
                                stop=(j == len(plist) - 1),
                                tile_position=(96, 0) if pb == 96 else None,
                            )
                        pos.append(po)
                    osb = spool.tile([128, SH], F32, name=f"osb{hc}", tag="osb", bufs=2)
                    nc.vector.tensor_scalar_add(osb[0:hw, :], pos[0][0:hw, :], bo_sb[hc][:])
                    for po in pos[1:]:
                        nc.vector.tensor_add(osb[0:hw, :], osb[0:hw, :], po[0:hw, :])
                    nc.sync.dma_start(out=outT[b, h0 : h0 + hw, hsl], in_=osb[0:hw, :])

    nc.compile()
    return nc


import ml_dtypes

BF16NP = ml_dtypes.bfloat16


def _pad_w(W):
    f32 = np.float32
    return np.ascontiguousarray(
        np.concatenate([W.astype(f32, copy=False), np.zeros((HID, 1), f32)], axis=1)
    ).astype(BF16NP)


def _prep_core_inputs(hidden_states, Wq, bq, Wk, bk, Wv, bv, Wo, bo):
    """Host-side layout prep (transpose/reorder/pad only, no math)."""
    f32 = np.float32
    hs = np.ascontiguousarray(hidden_states.astype(f32, copy=False))
    Wo_p = np.zeros((NPT, 128, HID2), dtype=BF16NP)
    for seg, off, ln, pt, pb in PIECES:
        g0 = BOUNDS[seg] + off
        Wo_p[pt, pb : pb + ln, :HID] = Wo[g0 : g0 + ln, :].astype(BF16NP)
    bvb = np.broadcast_to(bv.astype(f32, copy=False), (128, HID)).copy()
    ind9 = np.zeros((NSEG, NSEG * 128), dtype=f32)
    for sg in range(NSEG):
        ind9[sg, sg * 128 : sg * 128 + DSEG[sg]] = 1.0
    shared = {
        "Wq": _pad_w(Wq),
        "Wk": _pad_w(Wk),
        "Wv": _pad_w(Wv),
        "Wop": Wo_p,
        "bq": np.ascontiguousarray(bq.astype(f32, copy=False).reshape(HID, 1)),
        "bk": np.ascontiguousarray(bk.astype(f32, copy=False).reshape(HID, 1)),
        "bo": np.ascontiguousarray(bo.astype(f32, copy=False).reshape(HID, 1)),
        "bvb": bvb,
        "ind9": ind9,
    }
    in_maps = []
    for c in range(N_CORES):
        shard = hs[c * BPC : (c + 1) * BPC]
        m = dict(shared)
        m["hsT"] = np.ascontiguousarray(shard.transpose(0, 2, 1).astype(BF16NP))
        in_maps.append(m)
    return in_maps


LAST_RESULTS = None


def kernel(hidden_states, Wq, bq, Wk, bk, Wv, bv, Wo, bo):
    global LAST_RESULTS
    if "nc" not in _CACHE:
        _CACHE["nc"] = _build()
    nc = _CACHE["nc"]
    in_maps = _prep_core_inputs(hidden_states, Wq, bq, Wk, bk, Wv, bv, Wo, bo)
    kwargs = {}
    if os.environ.get("KERNEL_TRACE") == "1":
        kwargs["trace"] = True
        td = os.environ.get("KERNEL_TRACE_DIR")
        if td:
            kwargs["tmpdir"] = td
    res = run_bass_kernel_spmd(nc, in_maps, core_ids=list(range(N_CORES)), **kwargs)
    LAST_RESULTS = res
    out = np.empty((B, S, HID), dtype=np.float32)
    for c in range(N_CORES):
        out[c * BPC : (c + 1) * BPC] = res.results[c]["outT"].transpose(0, 2, 1)
    return out



# revision 2
# speedup vs baseline: 1.3352x; 1.3352x over previous
"""Trainium2 Bass kernel for segmented ("sparse") attention — v2.

Reference math (per batch of 16, S=1024, H=441):
  q = hs @ Wq + bq ; k = hs @ Wk + bk ; v = hs @ Wv + bv
  for each of 9 column segments [s,e): softmax(q_i k_i^T / sqrt(d_i)) @ v_i
  out = concat(ctx_i) @ Wo + bo

Sharding: pure data parallel over batch, 2 batches per NeuronCore x 8 cores.

v2 strategy (vs v1 baseline):
  - q/k projected DIRECTLY into the packed row layout (Wq/Wk columns
    pre-permuted on host) — eliminates the SBUF->SBUF repack DMAs.
  - v projected directly into the augmented layout (Wv columns
    pre-permuted with gaps; ones come from the bias tile) — eliminates
    per-segment DMAs + memsets.
  - scores/pv/out-proj matmuls are issued interleaved across the two
    segments sharing a 128-row pack tile: their PE row/col strips are
    disjoint, so the 32x32-tiled PE array runs them CONCURRENTLY.
  - pv accumulates both segments of a pack tile into ONE PSUM bank at
    disjoint partition (column-strip) offsets -> uT lands directly in
    packed layout, denominator row included (aug ones column).
  - normalization via per-pack-tile indicator matmul broadcast.
  - dense back-to-back PE work keeps the HAM clock gate at 8/8
    (2.4 GHz) instead of oscillating to 4/8.
"""

import os
import math
import numpy as np
from contextlib import ExitStack

import concourse.bacc as bacc
import concourse.tile as tile
import concourse.mybir as mybir
from concourse.bass_utils import run_bass_kernel_spmd

F32 = mybir.dt.float32
BF16 = mybir.dt.bfloat16
AF = mybir.ActivationFunctionType

HID = 441
HID2 = HID + 1
S = 1024
SH = 512
B = 16
N_CORES = 8
BPC = B // N_CORES
BOUNDS = [0, 7, 21, 49, 105, 161, 217, 273, 357, 441]
NSEG = 9
DSEG = [BOUNDS[i + 1] - BOUNDS[i] for i in range(NSEG)]
NHC = 4
HCH = [(i * 128, min(128, HID - i * 128)) for i in range(NHC)]
NTC = 8
NPT = 5

# Packed row layout: (seg, length, pack_tile, pack_base). One piece per
# segment; bases 32-aligned so segments sharing a tile occupy disjoint
# 32-row strips of the PE array (concurrent tiled matmuls).
PIECES = [
    (0, 7, 4, 0),
    (1, 14, 1, 96),
    (2, 28, 0, 96),
    (3, 56, 2, 0),
    (4, 56, 3, 0),
    (5, 56, 2, 64),
    (6, 56, 3, 64),
    (7, 84, 0, 0),
    (8, 84, 1, 0),
]
SEG_PIECE = {p[0]: p for p in PIECES}
# segments per pack tile, in emission order
PT_SEGS = [[] for _ in range(NPT)]
for seg, ln, pt, pb in PIECES:
    PT_SEGS[pt].append((seg, ln, pb))

# v augmented layout: per segment [v columns (d), ones column]
AUG_OFF = [BOUNDS[i] + i for i in range(NSEG)]
AUG_W = HID + NSEG  # 450

# out-projection accumulation groups: group 0 = base-0 pieces (their
# row strips all include strip 0 -> serialize among themselves), group 1
# = base-64 + base-96 pieces (all include strip 3). The two groups use
# separate PSUM banks so they can run concurrently.
WO_G0 = [p for p in PIECES if p[3] == 0]
WO_G1 = [p for p in PIECES if p[3] in (64, 96)]
# interleave for concurrency: pair g0[i] with g1[i]
WO_ORDER = []
for i in range(max(len(WO_G0), len(WO_G1))):
    if i < len(WO_G0):
        WO_ORDER.append((WO_G0[i], 0, i == 0, i == len(WO_G0) - 1))
    if i < len(WO_G1):
        WO_ORDER.append((WO_G1[i], 1, i == 0, i == len(WO_G1) - 1))

_CACHE = {}


def _build():
    nc = bacc.Bacc("TRN2", target_bir_lowering=False, debug=False)

    hsT = nc.dram_tensor("hsT", [BPC, HID, S], BF16, kind="ExternalInput").ap()
    Wqp_d = nc.dram_tensor("Wqp", [HID, NPT * 128], BF16, kind="ExternalInput").ap()
    Wkp_d = nc.dram_tensor("Wkp", [HID, NPT * 128], BF16, kind="ExternalInput").ap()
    Wva_d = nc.dram_tensor("Wva", [HID, AUG_W], BF16, kind="ExternalInput").ap()
    Wop_d = nc.dram_tensor("Wop", [NPT, 128, HID2], BF16, kind="ExternalInput").ap()
    bqp_d = nc.dram_tensor("bqp", [NPT, 128, 1], F32, kind="ExternalInput").ap()
    bkp_d = nc.dram_tensor("bkp", [NPT, 128, 1], F32, kind="ExternalInput").ap()
    bva_d = nc.dram_tensor("bva", [128, AUG_W], F32, kind="ExternalInput").ap()
    bo_d = nc.dram_tensor("bo", [HID, 1], F32, kind="ExternalInput").ap()
    indp_d = nc.dram_tensor("indp", [NPT, NSEG, 128], BF16, kind="ExternalInput").ap()
    outT = nc.dram_tensor("outT", [BPC, HID, S], F32, kind="ExternalOutput").ap()

    with tile.TileContext(nc) as tc, ExitStack() as ctx, nc.allow_low_precision(
        reason="bf16 matmuls + bf16 softmax intermediates"
    ):
        cpool = ctx.enter_context(tc.tile_pool(name="c", bufs=1))
        hpool = ctx.enter_context(tc.tile_pool(name="h", bufs=1))
        qkpool = ctx.enter_context(tc.tile_pool(name="qk", bufs=1))
        vpool = ctx.enter_context(tc.tile_pool(name="v", bufs=1))
        epool = ctx.enter_context(tc.tile_pool(name="e", bufs=4))
        upool = ctx.enter_context(tc.tile_pool(name="u", bufs=1))
        dpool = ctx.enter_context(tc.tile_pool(name="d", bufs=2))
        cxpool = ctx.enter_context(tc.tile_pool(name="cx", bufs=1))
        opool = ctx.enter_context(tc.tile_pool(name="o", bufs=2))
        ps_sc = ctx.enter_context(tc.tile_pool(name="psc", bufs=2, space="PSUM"))
        ps_pu = ctx.enter_context(tc.tile_pool(name="ppu", bufs=2, space="PSUM"))
        ps_x = ctx.enter_context(tc.tile_pool(name="px", bufs=2, space="PSUM"))

        # ---- constants ----
        Wq_sb, Wk_sb, Wv_sb, bo_sb = [], [], [], []
        for hc, (h0, hw) in enumerate(HCH):
            for lst, src, nm, w in (
                (Wq_sb, Wqp_d, "wq", NPT * 128),
                (Wk_sb, Wkp_d, "wk", NPT * 128),
                (Wv_sb, Wva_d, "wv", AUG_W),
            ):
                t = cpool.tile([hw, w], BF16, name=f"{nm}{hc}", tag=f"{nm}{hc}")
                nc.sync.dma_start(out=t, in_=src[h0 : h0 + hw, :])
                lst.append(t)
            t = cpool.tile([hw, 1], F32, name=f"bo{hc}", tag=f"bo{hc}")
            nc.sync.dma_start(out=t, in_=bo_d[h0 : h0 + hw, :])
            bo_sb.append(t)
        Wo_sb, bq_sb, bk_sb, ind_sb = [], [], [], []
        for i in range(NPT):
            t = cpool.tile([128, HID2], BF16, name=f"wo{i}", tag=f"wo{i}")
            nc.sync.dma_start(out=t, in_=Wop_d[i])
            Wo_sb.append(t)
            t = cpool.tile([128, 1], F32, name=f"bq{i}", tag=f"bq{i}")
            nc.sync.dma_start(out=t, in_=bqp_d[i])
            bq_sb.append(t)
            t = cpool.tile([128, 1], F32, name=f"bk{i}", tag=f"bk{i}")
            nc.sync.dma_start(out=t, in_=bkp_d[i])
            bk_sb.append(t)
            t = cpool.tile([NSEG, 128], BF16, name=f"ind{i}", tag=f"ind{i}")
            nc.sync.dma_start(out=t, in_=indp_d[i])
            ind_sb.append(t)
        bva_sb = cpool.tile([128, AUG_W], F32, name="bva", tag="bva")
        nc.sync.dma_start(out=bva_sb, in_=bva_d)

        for b in range(BPC):
            # ---- load hsT ----
            hs = []
            for hc, (h0, hw) in enumerate(HCH):
                t = hpool.tile([hw, S], BF16, name=f"hs{hc}", tag=f"hs{hc}", bufs=2)
                nc.sync.dma_start(out=t, in_=hsT[b, h0 : h0 + hw, :])
                hs.append(t)

            # ---- q/k projections directly into packed layout ----
            packs = {}
            for nm, W_sb, b_sb in (("q", Wq_sb, bq_sb), ("k", Wk_sb, bk_sb)):
                pk = []
                for pt in range(NPT):
                    qk = qkpool.tile(
                        [128, S], BF16, name=f"{nm}T{pt}", tag=f"{nm}T{pt}", bufs=2
                    )
                    for half in range(2):
                        pa = ps_x.tile([128, SH], F32, name=f"pp{nm}{pt}{half}", tag="x")
                        for hc, (h0, hw) in enumerate(HCH):
                            nc.tensor.matmul(
                                pa[:],
                                W_sb[hc][:, pt * 128 : (pt + 1) * 128],
                                hs[hc][:, half * SH : (half + 1) * SH],
                                start=(hc == 0),
                                stop=(hc == NHC - 1),
                            )
                        nc.vector.tensor_scalar_add(
                            qk[:, half * SH : (half + 1) * SH], pa[:], b_sb[pt][:]
                        )
                    pk.append(qk)
                packs[nm] = pk
            qT, kT = packs["q"], packs["k"]

            # ---- v projection directly into augmented layout ----
            vaug = []
            for sc in range(NTC):
                pv = ps_x.tile([128, AUG_W], F32, name=f"pv{sc}", tag="x")
                for hc, (h0, hw) in enumerate(HCH):
                    nc.tensor.matmul(
                        pv[:],
                        hs[hc][:, sc * 128 : (sc + 1) * 128],
                        Wv_sb[hc][:],
                        start=(hc == 0),
                        stop=(hc == NHC - 1),
                    )
                va = vpool.tile([128, AUG_W], BF16, name=f"va{sc}", tag=f"va{sc}", bufs=2)
                nc.vector.tensor_add(va[:], pv[:], bva_sb[:])
                vaug.append(va)

            # ---- attention ----
            cxT = [
                cxpool.tile([128, S], BF16, name=f"cxT{i}", tag=f"cxT{i}", bufs=2)
                for i in range(NPT)
            ]
            for half in range(2):
                hsl = slice(half * SH, (half + 1) * SH)
                den9 = dpool.tile([NSEG, SH], F32, name="den9", tag="den9")
                u_sb = {}
                for pt in range(NPT):
                    segs = PT_SEGS[pt]
                    # scores + exp: per t-chunk pair, both segments of the
                    # tile back-to-back (disjoint PE row strips -> overlap)
                    E = {}
                    for seg, ln, pb in segs:
                        E[seg] = epool.tile(
                            [128, NTC * SH], BF16, name=f"E{seg}", tag="E"
                        )
                    for t2 in range(NTC // 2):
                        pms = {}
                        for seg, ln, pb in segs:
                            pm = ps_sc.tile(
                                [128, 2 * SH], F32, name=f"pm{seg}{t2}", tag="sc"
                            )
                            pms[seg] = pm
                        for k2 in range(2):
                            t = 2 * t2 + k2
                            for seg, ln, pb in segs:
                                tp = (pb, 0) if pb else None
                                nc.tensor.matmul(
                                    pms[seg][:, k2 * SH : (k2 + 1) * SH],
                                    kT[pt][pb : pb + ln, t * 128 : (t + 1) * 128],
                                    qT[pt][pb : pb + ln, hsl],
                                    start=True,
                                    stop=True,
                                    tile_position=tp,
                                )
                        for seg, ln, pb in segs:
                            scale = 1.0 / math.sqrt(ln)
                            nc.scalar.activation(
                                E[seg][:, t2 * 2 * SH : (t2 + 1) * 2 * SH],
                                pms[seg][:],
                                AF.Exp,
                                scale=scale,
                            )
                    # pv: both segments accumulate into ONE bank at their
                    # packed column strips (disjoint partitions)
                    pu = ps_pu.tile([128, SH], F32, name=f"pu{pt}", tag="pu")
                    for t in range(NTC):
                        for seg, ln, pb in segs:
                            a0 = AUG_OFF[seg]
                            nc.tensor.matmul(
                                pu[pb : pb + ln + 1, :],
                                vaug[t][:, a0 : a0 + ln + 1],
                                E[seg][:, t * SH : (t + 1) * SH],
                                start=(t == 0),
                                stop=(t == NTC - 1),
                                tile_position=(0, pb),
                                skip_group_check=True,
                            )
                    u = upool.tile([128, SH], BF16, name=f"u{pt}", tag=f"u{pt}", bufs=2)
                    nc.vector.tensor_copy(u[:], pu[:])
                    u_sb[pt] = u
                    for seg, ln, pb in segs:
                        nc.gpsimd.dma_start(
                            out=den9[seg : seg + 1, :], in_=u[pb + ln : pb + ln + 1, :]
                        )
                # normalization
                rec9 = dpool.tile([NSEG, SH], F32, name="rec9", tag="rec9")
                scr9 = dpool.tile([NSEG, SH], F32, name="scr9", tag="scr9")
                nc.vector.reciprocal_approx_accurate(rec9[:], den9[:], scratch=scr9[:])
                rec9b = dpool.tile([NSEG, SH], BF16, name="rec9b", tag="rec9b")
                nc.vector.tensor_copy(rec9b[:], rec9[:])
                for pt in range(NPT):
                    recb = ps_x.tile([128, SH], F32, name=f"rb{pt}", tag="x")
                    nc.tensor.matmul(
                        recb[:], ind_sb[pt][:], rec9b[:], start=True, stop=True
                    )
                    nc.vector.tensor_mul(cxT[pt][:, hsl], u_sb[pt][:], recb[:])

                # ---- output projection ----
                for hc, (h0, hw) in enumerate(HCH):
                    po = [
                        ps_x.tile([128, SH], F32, name=f"po{hc}{g}", tag="x")
                        for g in range(2)
                    ]
                    for (seg, ln, pt, pb), g, first, last in WO_ORDER:
                        tp = (pb, 0) if pb else None
                        nc.tensor.matmul(
                            po[g][0:hw, :],
                            Wo_sb[pt][pb : pb + ln, h0 : h0 + hw],
                            cxT[pt][pb : pb + ln, hsl],
                            start=first,
                            stop=last,
                            tile_position=tp,
                            skip_group_check=True,
                        )
                    osb = opool.tile([128, SH], F32, name=f"osb{hc}", tag="osb")
                    nc.vector.tensor_scalar_add(osb[0:hw, :], po[0][0:hw, :], bo_sb[hc][:])
                    nc.vector.tensor_add(osb[0:hw, :], osb[0:hw, :], po[1][0:hw, :])
                    nc.sync.dma_start(out=outT[b, h0 : h0 + hw, hsl], in_=osb[0:hw, :])

    nc.compile()
    return nc


import ml_dtypes

BF16NP = ml_dtypes.bfloat16


def _prep_core_inputs(hidden_states, Wq, bq, Wk, bk, Wv, bv, Wo, bo):
    """Host-side layout prep (transpose/reorder/pad only, no math)."""
    f32 = np.float32
    hs = np.ascontiguousarray(hidden_states.astype(f32, copy=False))
    Wq = np.asarray(Wq, dtype=f32)
    Wk = np.asarray(Wk, dtype=f32)
    Wv = np.asarray(Wv, dtype=f32)
    Wo = np.asarray(Wo, dtype=f32)
    bq = np.asarray(bq, dtype=f32)
    bk = np.asarray(bk, dtype=f32)
    bv = np.asarray(bv, dtype=f32)
    bo = np.asarray(bo, dtype=f32)

    Wqp = np.zeros((HID, NPT * 128), dtype=BF16NP)
    Wkp = np.zeros((HID, NPT * 128), dtype=BF16NP)
    bqp = np.zeros((NPT, 128, 1), dtype=f32)
    bkp = np.zeros((NPT, 128, 1), dtype=f32)
    Wop = np.zeros((NPT, 128, HID2), dtype=BF16NP)
    indp = np.zeros((NPT, NSEG, 128), dtype=BF16NP)
    for seg, ln, pt, pb in PIECES:
        g0 = BOUNDS[seg]
        Wqp[:, pt * 128 + pb : pt * 128 + pb + ln] = Wq[:, g0 : g0 + ln].astype(BF16NP)
        Wkp[:, pt * 128 + pb : pt * 128 + pb + ln] = Wk[:, g0 : g0 + ln].astype(BF16NP)
        bqp[pt, pb : pb + ln, 0] = bq[g0 : g0 + ln]
        bkp[pt, pb : pb + ln, 0] = bk[g0 : g0 + ln]
        Wop[pt, pb : pb + ln, :HID] = Wo[g0 : g0 + ln, :].astype(BF16NP)
        indp[pt, seg, pb : pb + ln] = 1.0

    Wva = np.zeros((HID, AUG_W), dtype=BF16NP)
    bva = np.zeros((128, AUG_W), dtype=f32)
    for sg in range(NSEG):
        s0, s1 = BOUNDS[sg], BOUNDS[sg + 1]
        a0 = AUG_OFF[sg]
        Wva[:, a0 : a0 + (s1 - s0)] = Wv[:, s0:s1].astype(BF16NP)
        bva[:, a0 : a0 + (s1 - s0)] = bv[s0:s1]
        bva[:, a0 + (s1 - s0)] = 1.0

    shared = {
        "Wqp": Wqp,
        "Wkp": Wkp,
        "Wva": Wva,
        "Wop": Wop,
        "bqp": bqp,
        "bkp": bkp,
        "bva": bva,
        "bo": np.ascontiguousarray(bo.reshape(HID, 1)),
        "indp": indp,
    }
    in_maps = []
    for c in range(N_CORES):
        shard = hs[c * BPC : (c + 1) * BPC]
        m = dict(shared)
        m["hsT"] = np.ascontiguousarray(shard.transpose(0, 2, 1).astype(BF16NP))
        in_maps.append(m)
    return in_maps


LAST_RESULTS = None


def kernel(hidden_states, Wq, bq, Wk, bk, Wv, bv, Wo, bo):
    global LAST_RESULTS
    if "nc" not in _CACHE:
        _CACHE["nc"] = _build()
    nc = _CACHE["nc"]
    in_maps = _prep_core_inputs(hidden_states, Wq, bq, Wk, bk, Wv, bv, Wo, bo)
    kwargs = {}
    if os.environ.get("KERNEL_TRACE") == "1":
        kwargs["trace"] = True
        td = os.environ.get("KERNEL_TRACE_DIR")
        if td:
            kwargs["tmpdir"] = td
    res = run_bass_kernel_spmd(nc, in_maps, core_ids=list(range(N_CORES)), **kwargs)
    LAST_RESULTS = res
    out = np.empty((B, S, HID), dtype=np.float32)
    for c in range(N_CORES):
        out[c * BPC : (c + 1) * BPC] = res.results[c]["outT"].transpose(0, 2, 1)
    return out


# revision 6
# speedup vs baseline: 1.4535x; 1.0887x over previous
"""Trainium2 Bass kernel for segmented attention — v3.

Key ideas vs v2:
  - TWO independent packed layouts: qT/kT pack segments whole (scores
    stay single-segment matmuls), arranged so concurrent waves use
    disjoint 32-row PE strips; the ctx/pu/Wo side SPLITS the two 84-row
    segments into 56+28 pieces so no pv/out-proj matmul rounds its PE
    tile to 128 columns (which would hog all four strips).
  - pv: all pieces of a pack tile accumulate into ONE PSUM bank at
    disjoint partition strips -> concurrent tiled matmuls.
  - biases folded into matmuls via an all-ones row appended to hsT
    (weight row 441 = bias); output bias rides on segment 0's
    denominator row of cxT (den * recip ~= 1).
  - startup DMAs spread across five engine queues; batch-1 projections
    interleaved into batch-0 attention so PE has filler while ACT
    grinds exps.
"""

import os
import math
import numpy as np
from contextlib import ExitStack

import concourse.bacc as bacc
import concourse.tile as tile
import concourse.mybir as mybir
from concourse.bass_utils import run_bass_kernel_spmd

F32 = mybir.dt.float32
BF16 = mybir.dt.bfloat16
AF = mybir.ActivationFunctionType

HID = 441
HIDA = HID + 1  # +1 ones row for bias folding
HID2 = HID + 1  # Wo free-dim pad to even
S = 1024
SH = 512
B = 16
N_CORES = 8
BPC = B // N_CORES
BOUNDS = [0, 7, 21, 49, 105, 161, 217, 273, 357, 441]
NSEG = 9
DSEG = [BOUNDS[i + 1] - BOUNDS[i] for i in range(NSEG)]
NHC = 4
HCH_IN = [(i * 128, min(128, HIDA - i * 128)) for i in range(NHC)]  # 442 rows
HCH_OUT = [(i * 128, min(128, HID - i * 128)) for i in range(NHC)]  # 441 rows
NTC = 8
NPT = 5

# ---- scores-side packing of q/k rows: whole segments ----
# seg -> (pack_tile, base). Waves pair segments with disjoint strips.
SC_PACK = {
    7: (0, 0),
    8: (1, 0),
    3: (2, 0),
    5: (2, 64),
    4: (3, 0),
    6: (3, 64),
    2: (4, 0),
    1: (4, 32),
    0: (4, 64),
}
SC_WAVES = [[7], [8], [3, 5], [4, 6], [2, 1], [0]]

# ---- ctx-side packing: pieces (name, seg, src_off, ln, pt, pb, has_den) ----
PIECES = [
    ("A7", 7, 0, 56, 0, 0, False),
    ("s5", 5, 0, 56, 0, 64, True),
    ("A8", 8, 0, 56, 1, 0, False),
    ("s6", 6, 0, 56, 1, 64, True),
    ("s3", 3, 0, 56, 2, 0, True),
    ("s4", 4, 0, 56, 2, 64, True),
    ("B7", 7, 56, 28, 3, 0, True),
    ("B8", 8, 56, 28, 3, 64, True),
    ("s2", 2, 0, 28, 3, 96, True),
    ("s1", 1, 0, 14, 4, 64, True),
    ("s0", 0, 0, 7, 4, 0, True),
]
PBYN = {p[0]: p for p in PIECES}
# col-strip base 32 is avoided everywhere: matmuls with tile_position
# (0, 32) produce garbage on this hardware (col quadrant 1 bug).
PV_TILES = [["A7", "s5"], ["A8", "s6"], ["s3", "s4"], ["B7", "B8", "s2"], ["s0", "s1"]]

# out-projection: two concurrent accumulation groups (two PSUM banks).
# Within a group every CONSECUTIVE pair overlaps PE row strips so the
# chain serializes in hardware (disjoint-strip same-bank accumulation
# would race). s0 uses ln+1 rows: its denominator row of cxT is ~1.0
# and Wo row 7 carries the output bias.
WO_G0 = ["A7", "A8", "s3", "B7", "s0"]
WO_G1 = ["s5", "B8", "s6", "s2", "s4", "s1"]
WO_ORDER = []
for i in range(max(len(WO_G0), len(WO_G1))):
    if i < len(WO_G0):
        WO_ORDER.append((WO_G0[i], 0, i == 0, i == len(WO_G0) - 1))
    if i < len(WO_G1):
        WO_ORDER.append((WO_G1[i], 1, i == 0, i == len(WO_G1) - 1))

AUG_OFF = [BOUNDS[i] + i for i in range(NSEG)]
AUG_W = HID + NSEG  # 450

_CACHE = {}


def _build():
    nc = bacc.Bacc("TRN2", target_bir_lowering=False, debug=False)

    hsT = nc.dram_tensor("hsT", [BPC, HIDA, S], BF16, kind="ExternalInput").ap()
    Wqp_d = nc.dram_tensor("Wqp", [HIDA, NPT * 128], BF16, kind="ExternalInput").ap()
    Wkp_d = nc.dram_tensor("Wkp", [HIDA, NPT * 128], BF16, kind="ExternalInput").ap()
    Wva_d = nc.dram_tensor("Wva", [HIDA, AUG_W], BF16, kind="ExternalInput").ap()
    Wop_d = nc.dram_tensor("Wop", [NPT, 128, HID2], BF16, kind="ExternalInput").ap()
    indp_d = nc.dram_tensor("indp", [NPT, NSEG, 128], BF16, kind="ExternalInput").ap()
    outT = nc.dram_tensor("outT", [BPC, HID, S], F32, kind="ExternalOutput").ap()

    with tile.TileContext(nc) as tc, ExitStack() as ctx, nc.allow_low_precision(
        reason="bf16 matmuls + bf16 softmax intermediates"
    ):
        cpool = ctx.enter_context(tc.tile_pool(name="c", bufs=1))
        hpool = ctx.enter_context(tc.tile_pool(name="h", bufs=1))
        qkpool = ctx.enter_context(tc.tile_pool(name="qk", bufs=1))
        vpool = ctx.enter_context(tc.tile_pool(name="v", bufs=1))
        epool = ctx.enter_context(tc.tile_pool(name="e", bufs=6))
        upool = ctx.enter_context(tc.tile_pool(name="u", bufs=1))
        dpool = ctx.enter_context(tc.tile_pool(name="d", bufs=2))
        cxpool = ctx.enter_context(tc.tile_pool(name="cx", bufs=1))
        opool = ctx.enter_context(tc.tile_pool(name="o", bufs=2))
        ps_sc = ctx.enter_context(tc.tile_pool(name="psc", bufs=2, space="PSUM"))
        ps_pu = ctx.enter_context(tc.tile_pool(name="ppu", bufs=2, space="PSUM"))
        ps_x = ctx.enter_context(tc.tile_pool(name="px", bufs=2, space="PSUM"))

        # ---- constants, spread across DMA queues for parallel startup ----
        Wq_sb, Wk_sb, Wv_sb = [], [], []
        for hc, (h0, hw) in enumerate(HCH_IN):
            t = cpool.tile([hw, NPT * 128], BF16, name=f"wq{hc}", tag=f"wq{hc}")
            nc.gpsimd.dma_start(out=t, in_=Wqp_d[h0 : h0 + hw, :])
            Wq_sb.append(t)
            t = cpool.tile([hw, NPT * 128], BF16, name=f"wk{hc}", tag=f"wk{hc}")
            nc.scalar.dma_start(out=t, in_=Wkp_d[h0 : h0 + hw, :])
            Wk_sb.append(t)
            t = cpool.tile([hw, AUG_W], BF16, name=f"wv{hc}", tag=f"wv{hc}")
            nc.gpsimd.dma_start(out=t, in_=Wva_d[h0 : h0 + hw, :])
            Wv_sb.append(t)
        Wo_sb, ind_sb = [], []
        for i in range(NPT):
            t = cpool.tile([128, HID2], BF16, name=f"wo{i}", tag=f"wo{i}")
            nc.scalar.dma_start(out=t, in_=Wop_d[i])
            Wo_sb.append(t)
            t = cpool.tile([NSEG, 128], BF16, name=f"ind{i}", tag=f"ind{i}")
            nc.gpsimd.dma_start(out=t, in_=indp_d[i])
            ind_sb.append(t)

        hs_all = {}

        def load_hs(b):
            hs = []
            for hc, (h0, hw) in enumerate(HCH_IN):
                t = hpool.tile([hw, S], BF16, name=f"hs{hc}", tag=f"hs{hc}", bufs=2)
                nc.sync.dma_start(out=t, in_=hsT[b, h0 : h0 + hw, :])
                hs.append(t)
            hs_all[b] = hs

        qk_all = {}

        def emit_qkproj(b, pt):
            """q and k projection for one scores pack tile of batch b."""
            if b not in qk_all:
                qk_all[b] = {"q": [None] * NPT, "k": [None] * NPT}
            hs = hs_all[b]
            for nm, W_sb in (("q", Wq_sb), ("k", Wk_sb)):
                qk = qkpool.tile(
                    [128, S], BF16, name=f"{nm}T{pt}", tag=f"{nm}T{pt}", bufs=2
                )
                for half in range(2):
                    pa = ps_x.tile([128, SH], F32, name=f"pp{nm}{pt}{half}", tag="x")
                    for hc, (h0, hw) in enumerate(HCH_IN):
                        nc.tensor.matmul(
                            pa[:],
                            W_sb[hc][:, pt * 128 : (pt + 1) * 128],
                            hs[hc][:, half * SH : (half + 1) * SH],
                            start=(hc == 0),
                            stop=(hc == NHC - 1),
                        )
                    nc.vector.tensor_copy(qk[:, half * SH : (half + 1) * SH], pa[:])
                qk_all[b][nm][pt] = qk

        va_all = {}

        def emit_vproj(b, sc):
            if b not in va_all:
                va_all[b] = [None] * NTC
            hs = hs_all[b]
            pv = ps_x.tile([128, AUG_W], F32, name=f"pv{sc}", tag="x")
            for hc, (h0, hw) in enumerate(HCH_IN):
                nc.tensor.matmul(
                    pv[:],
                    hs[hc][:, sc * 128 : (sc + 1) * 128],
                    Wv_sb[hc][:],
                    start=(hc == 0),
                    stop=(hc == NHC - 1),
                )
            va = vpool.tile([128, AUG_W], BF16, name=f"va{sc}", tag=f"va{sc}", bufs=2)
            nc.vector.tensor_copy(va[:], pv[:])
            va_all[b][sc] = va

        cx_all = {}

        def emit_attention_half(b, half, filler=None):
            """One query-half of attention for batch b. filler(i) is called
            between scores waves to interleave independent PE work."""
            qT, kT = qk_all[b]["q"], qk_all[b]["k"]
            vaug = va_all[b]
            if b not in cx_all:
                cx_all[b] = [
                    cxpool.tile([128, S], BF16, name=f"cx{b}{i}", tag=f"cxT{i}", bufs=2)
                    for i in range(NPT)
                ]
            cxT = cx_all[b]
            hsl = slice(half * SH, (half + 1) * SH)
            den9 = dpool.tile([NSEG, SH], F32, name="den9", tag="den9")

            # ---- scores + exp, wave order ----
            E = {}
            for wi, wave in enumerate(SC_WAVES):
                for seg in wave:
                    E[seg] = epool.tile([128, NTC * SH], BF16, name=f"E{seg}", tag="E")
                for t2 in range(NTC // 2):
                    pms = {
                        seg: ps_sc.tile([128, 2 * SH], F32, name=f"pm{seg}{t2}", tag="sc")
                        for seg in wave
                    }
                    for k2 in range(2):
                        t = 2 * t2 + k2
                        for seg in wave:
                            pt, pb = SC_PACK[seg]
                            d = DSEG[seg]
                            nc.tensor.matmul(
                                pms[seg][:, k2 * SH : (k2 + 1) * SH],
                                kT[pt][pb : pb + d, t * 128 : (t + 1) * 128],
                                qT[pt][pb : pb + d, hsl],
                                start=True,
                                stop=True,
                                tile_position=(pb, 0) if pb else None,
                            )
                    for seg in wave:
                        nc.scalar.activation(
                            E[seg][:, t2 * 2 * SH : (t2 + 1) * 2 * SH],
                            pms[seg][:],
                            AF.Exp,
                            scale=1.0 / math.sqrt(DSEG[seg]),
                        )
                if filler is not None:
                    filler(wi)

            # ---- pv per ctx pack tile ----
            u_sb = [None] * NPT
            for pt, tiles in enumerate(PV_TILES):
                pu = ps_pu.tile([128, SH], F32, name=f"pu{pt}", tag="pu")
                for t in range(NTC):
                    for pn in tiles:
                        _, seg, off, ln, _, pb, has_den = PBYN[pn]
                        a0 = AUG_OFF[seg] + off
                        w = ln + 1 if has_den else ln
                        nc.tensor.matmul(
                            pu[pb : pb + w, :],
                            vaug[t][:, a0 : a0 + w],
                            E[seg][:, t * SH : (t + 1) * SH],
                            start=(t == 0),
                            stop=(t == NTC - 1),
                            tile_position=(0, pb),
                            skip_group_check=True,
                        )
                u = upool.tile([128, SH], BF16, name=f"u{pt}", tag=f"u{pt}", bufs=2)
                nc.vector.tensor_copy(u[:], pu[:])
                u_sb[pt] = u
                for pn in tiles:
                    _, seg, off, ln, _, pb, has_den = PBYN[pn]
                    if has_den:
                        nc.gpsimd.dma_start(
                            out=den9[seg : seg + 1, :], in_=u[pb + ln : pb + ln + 1, :]
                        )

            # ---- normalize ----
            rec9 = dpool.tile([NSEG, SH], F32, name="rec9", tag="rec9")
            scr9 = dpool.tile([NSEG, SH], F32, name="scr9", tag="scr9")
            nc.vector.reciprocal_approx_accurate(rec9[:], den9[:], scratch=scr9[:])
            rec9b = dpool.tile([NSEG, SH], BF16, name="rec9b", tag="rec9b")
            nc.vector.tensor_copy(rec9b[:], rec9[:])
            for pt in range(NPT):
                recb = ps_x.tile([128, SH], F32, name=f"rb{pt}", tag="x")
                nc.tensor.matmul(
                    recb[:], ind_sb[pt][:], rec9b[:], start=True, stop=True
                )
                nc.vector.tensor_mul(cxT[pt][:, hsl], u_sb[pt][:], recb[:])

            # ---- output projection ----
            for hc, (h0, hw) in enumerate(HCH_OUT):
                po = [
                    ps_x.tile([128, SH], F32, name=f"po{hc}{g}", tag="x")
                    for g in range(2)
                ]
                for pn, g, first, last in WO_ORDER:
                    _, seg, off, ln, pt, pb, _ = PBYN[pn]
                    w = ln + 1 if pn == "s0" else ln
                    nc.tensor.matmul(
                        po[g][0:hw, :],
                        Wo_sb[pt][pb : pb + w, h0 : h0 + hw],
                        cxT[pt][pb : pb + w, hsl],
                        start=first,
                        stop=last,
                        tile_position=(pb, 0) if pb else None,
                        skip_group_check=True,
                    )
                osb = opool.tile([128, SH], F32, name=f"osb{hc}", tag="osb")
                nc.vector.tensor_copy(osb[0:hw, :], po[0][0:hw, :])
                nc.vector.tensor_add(osb[0:hw, :], osb[0:hw, :], po[1][0:hw, :])
                nc.sync.dma_start(out=outT[b, h0 : h0 + hw, hsl], in_=osb[0:hw, :])

        # ================= emission schedule =================
        load_hs(0)
        for pt in range(NPT):
            emit_qkproj(0, pt)
        for sc in range(NTC):
            emit_vproj(0, sc)
        load_hs(1)

        def b1_proj_filler(wi):
            if wi < NPT:
                emit_qkproj(1, wi)

        emit_attention_half(0, 0, filler=b1_proj_filler)

        def b1_vproj_filler(wi):
            if wi < 4:
                emit_vproj(1, 2 * wi)
                emit_vproj(1, 2 * wi + 1)

        emit_attention_half(0, 1, filler=b1_vproj_filler)
        emit_attention_half(1, 0)
        emit_attention_half(1, 1)

    nc.compile()
    return nc


import ml_dtypes

BF16NP = ml_dtypes.bfloat16


def _prep_core_inputs(hidden_states, Wq, bq, Wk, bk, Wv, bv, Wo, bo):
    """Host-side layout prep (transpose/reorder/pad only, no math)."""
    f32 = np.float32
    hs = np.ascontiguousarray(hidden_states.astype(f32, copy=False))
    Wq = np.asarray(Wq, dtype=f32)
    Wk = np.asarray(Wk, dtype=f32)
    Wv = np.asarray(Wv, dtype=f32)
    Wo = np.asarray(Wo, dtype=f32)
    bq = np.asarray(bq, dtype=f32)
    bk = np.asarray(bk, dtype=f32)
    bv = np.asarray(bv, dtype=f32)
    bo = np.asarray(bo, dtype=f32)

    # scores-side q/k packing (whole segments)
    Wqp = np.zeros((HIDA, NPT * 128), dtype=f32)
    Wkp = np.zeros((HIDA, NPT * 128), dtype=f32)
    for seg, (pt, pb) in SC_PACK.items():
        g0, d = BOUNDS[seg], DSEG[seg]
        Wqp[:HID, pt * 128 + pb : pt * 128 + pb + d] = Wq[:, g0 : g0 + d]
        Wqp[HID, pt * 128 + pb : pt * 128 + pb + d] = bq[g0 : g0 + d]
        Wkp[:HID, pt * 128 + pb : pt * 128 + pb + d] = Wk[:, g0 : g0 + d]
        Wkp[HID, pt * 128 + pb : pt * 128 + pb + d] = bk[g0 : g0 + d]

    # ctx-side packing (split pieces)
    Wop = np.zeros((NPT, 128, HID2), dtype=BF16NP)
    indp = np.zeros((NPT, NSEG, 128), dtype=BF16NP)
    for pn, seg, off, ln, pt, pb, has_den in PIECES:
        g0 = BOUNDS[seg] + off
        Wop[pt, pb : pb + ln, :HID] = Wo[g0 : g0 + ln, :].astype(BF16NP)
        indp[pt, seg, pb : pb + ln + (1 if has_den else 0)] = 1.0
    Wop[4, 7, :HID] = bo.astype(BF16NP)  # rides on cxT's ~1.0 denom row

    Wva = np.zeros((HIDA, AUG_W), dtype=f32)
    for sg in range(NSEG):
        s0, s1 = BOUNDS[sg], BOUNDS[sg + 1]
        a0 = AUG_OFF[sg]
        Wva[:HID, a0 : a0 + (s1 - s0)] = Wv[:, s0:s1]
        Wva[HID, a0 : a0 + (s1 - s0)] = bv[s0:s1]
        Wva[HID, a0 + (s1 - s0)] = 1.0  # ones column for the denominator

    shared = {
        "Wqp": Wqp.astype(BF16NP),
        "Wkp": Wkp.astype(BF16NP),
        "Wva": Wva.astype(BF16NP),
        "Wop": Wop,
        "indp": indp,
    }
    in_maps = []
    for c in range(N_CORES):
        shard = hs[c * BPC : (c + 1) * BPC]
        hsA = np.ones((BPC, HIDA, S), dtype=BF16NP)
        hsA[:, :HID, :] = shard.transpose(0, 2, 1).astype(BF16NP)
        m = dict(shared)
        m["hsT"] = hsA
        in_maps.append(m)
    return in_maps


LAST_RESULTS = None


def kernel(hidden_states, Wq, bq, Wk, bk, Wv, bv, Wo, bo):
    global LAST_RESULTS
    if "nc" not in _CACHE:
        _CACHE["nc"] = _build()
    nc = _CACHE["nc"]
    in_maps = _prep_core_inputs(hidden_states, Wq, bq, Wk, bk, Wv, bv, Wo, bo)
    kwargs = {}
    if os.environ.get("KERNEL_TRACE") == "1":
        kwargs["trace"] = True
        td = os.environ.get("KERNEL_TRACE_DIR")
        if td:
            kwargs["tmpdir"] = td
    res = run_bass_kernel_spmd(nc, in_maps, core_ids=list(range(N_CORES)), **kwargs)
    LAST_RESULTS = res
    out = np.empty((B, S, HID), dtype=np.float32)
    for c in range(N_CORES):
        out[c * BPC : (c + 1) * BPC] = res.results[c]["outT"].transpose(0, 2, 1)
    return out
